# revision 28
# baseline (speedup 1.0000x reference)
"""Compact bilinear pooling kernel for 8 Trainium2 NeuronCores.

Algorithm (host side folds everything into matmul weights):
  out[b,:,n] = circconv_1024(S1 @ x1[b,:,n], S2 @ x2[b,:,n])
Decomposed via x^1024-1 = (x^512-1)(x^512+1):
  cyclic-512 branch (rFFT512) + negacyclic-512 branch (odd DFT), both fused
  with the count-sketch matrices into dense real forward matrices
  W_j [512c -> 1024 freq rows], applied as bf16 matmuls. Middle (complex
  multiply) runs on bf16 SBUF tiles on the vector engine. Inverse transforms
  are two block-diagonal [512 rows -> 512 outs] bf16 matmuls; the final
  unfold (c+d, c-d) is split between the vector and gpsimd engines reading
  inverse PSUM and writing packed bf16 output tiles per position tile.

v3 layout (current KCFG):
  - x1/x2 and all weights cast to bf16 on the HOST; x/weight loads are plain
    HWDGE transfers on the sync (SP) queue.
  - output staged in bf16 (host upcasts to f32 after gather; ~0.1% extra
    quantization, well inside the 2e-2 gate): halves store traffic and makes
    staged unfold ops pure-bf16 (2x DVE rate).
  - steady tiles: all 8 lo/hi outputs packed into ONE [128, 8*PT] bf16 tile,
    stored as a single SWDGE DMA on the Pool queue (25ns SEQ issue; descgen
    on the idle Pool engine) -> HWDGE/SP freed, DMA count per tile = 3.
  - unfold engine split: cs0/cs1 on Pool, rest on DVE; the 2 single-row
    DC/Nyquist cmult fixups on Pool (each costs a full row-time).
  - last tile: cs on Pool, act_d staging (Act drains psd -> no PE stall on
    PSUM banks; unfolds become 265ns bf16 ops), stores packed per-oc and
    spread across sync/scalar HWDGE queues to shorten the tail chain.
  - PE warm-up matmuls pin pe_busy_start early so real matmuls run at full
    p-state; head is DMA-pipeline-bound (x1 + first w piece ~4.7us).

Sharding: batch 32 -> 4 per core (data parallel), weights replicated.
Layout: channels/freq rows on SBUF partitions, positions on free axis.
No transposes anywhere.
"""
import sys

sys.path.insert(0, "/opt/trn_rl_repo")

import numpy as np
import ml_dtypes
import concourse.bass as bass
import concourse.mybir as mybir
from concourse import bacc
from concourse.tile import TileContext
from concourse.bass_utils import run_bass_kernel_spmd

B, C, HW, O = 32, 512, 784, 1024
NCORES = 8
BPC = B // NCORES  # 4 batches per core
PT = 392  # positions per tile (784 = 2*392; tiles never cross batch bounds)
NT = BPC * HW // PT  # 8 pos tiles per core
H = O // 2  # 512
F32, F32R, BF16 = mybir.dt.float32, mybir.dt.float32r, mybir.dt.bfloat16
BF16NP = ml_dtypes.bfloat16


def _build_host_matrices(sketch1, sketch2):
    """Fused fwd [512 c, 1024 freq-rows]; inverse IE/IF [256,256], ID [512,512].

    Level-2 folded row layout: e=rfft256, f=oddDFT256, d=oddDFT512; inverse
    weights carry the unfold 1/2 factors. All returned as bf16.
    """

    def build_fwd(sketch):
        sk = np.asarray(sketch, dtype=np.float64)
        Sp = sk[:H] + sk[H:]
        Sm = sk[:H] - sk[H:]
        Spp = Sp[:256] + Sp[256:]
        Spm = Sp[:256] - Sp[256:]
        n2 = np.arange(256)[None, :]
        k2 = np.arange(129)[:, None]
        Mc2 = np.exp(-2j * np.pi * k2 * n2 / 256) @ Spp
        k2f = np.arange(128)[:, None]
        Mo2 = np.exp(-2j * np.pi * n2 * (2 * k2f + 1) / 512) @ Spm
        n = np.arange(H)[None, :]
        ko = np.arange(256)[:, None]
        Mo = np.exp(-2j * np.pi * n * (2 * ko + 1) / O) @ Sm
        W = np.zeros((O, C))
        W[0:128] = Mc2[0:128].real
        W[128] = Mc2[128].real
        W[129:256] = Mc2[1:128].imag
        W[256:384] = Mo2.real
        W[384:512] = Mo2.imag
        W[512:768] = Mo.real
        W[768:1024] = Mo.imag
        return np.ascontiguousarray(W.T).astype(BF16NP)  # [C, O]

    j2 = np.arange(256)[None, :]
    k = np.arange(128)[:, None]
    IE = np.zeros((256, 256))
    IE[0:128] = 2 * np.cos(2 * np.pi * k * j2 / 256) / 256
    IE[0] = 1.0 / 256
    IE[128] = np.cos(np.pi * j2) / 256
    ki = np.arange(1, 128)[:, None]
    IE[129:256] = -2 * np.sin(2 * np.pi * ki * j2 / 256) / 256
    IF = np.zeros((256, 256))
    IF[0:128] = 2 * np.cos(2 * np.pi * (2 * k + 1) * j2 / 512) / 256
    IF[128:256] = -2 * np.sin(2 * np.pi * (2 * k + 1) * j2 / 512) / 256
    j = np.arange(H)[None, :]
    ko = np.arange(256)[:, None]
    ID = np.zeros((H, H))
    ID[0:256] = 2 * np.cos(2 * np.pi * (2 * ko + 1) * j / O) / H
    ID[256:512] = -2 * np.sin(2 * np.pi * (2 * ko + 1) * j / O) / H
    return (
        build_fwd(sketch1),
        build_fwd(sketch2),
        (IE / 4).astype(BF16NP),
        (IF / 4).astype(BF16NP),
        (ID / 2).astype(BF16NP),
    )


def _build_program(cfg=None):
    cfg = cfg or {}
    psf_bufs = cfg.get("psf_bufs", 2)
    pse_bufs = cfg.get("pse_bufs", 1)
    psq_bufs = cfg.get("psq_bufs", 1)
    psd_bufs = cfg.get("psd_bufs", 4)
    xbufs = cfg.get("xbufs", 2)
    fbufs = cfg.get("fbufs", 3)
    obufs = cfg.get("obufs", 2)
    # engine per unfold op [cs0..cs3, lo0,hi0,lo1,hi1,lo2,hi2,lo3,hi3]
    ueng = cfg.get("ueng", "ddpp" + "dp" * 4)
    store_split = cfg.get("store_split", 4)  # 1 or 4 pieces per tile
    warm = cfg.get("warm", 6)  # warm-up matmuls to ramp PE clock
    warm_ap = cfg.get("warm_ap", 392)
    x_eng = cfg.get("x_eng", "sync")  # HWDGE queue for x loads
    o_eng = cfg.get("o_eng", "scalar")  # queue for output stores
    es_skip = cfg.get("es_skip", False)  # cs reads pe_ PSUM directly
    tail_split = cfg.get("tail_split", False)
    x_first = cfg.get("x_first", True)  # lead DMA queues with tile-0 x loads
    nt_override = cfg.get("nt", NT)
    packed_oc = cfg.get("packed_oc", False)  # lo+hi in one tile, 1 DMA per oc
    act_f = cfg.get("act_f", None)  # drain pf_ PSUM->SBUF (needed for cs on pool)
    act_d = cfg.get("act_d", None)  # drain pd PSUM->SBUF (needed for lo/hi on pool)
    xsplit0 = cfg.get("xsplit0", False)  # split tile-0 x1 load per cc chunk
    warm_eng = cfg.get("warm_eng", "gpsimd")  # engine for warm tile memset
    # per-store queue pattern for the last tile (when not packed)
    store_eng_last = cfg.get("store_eng_last", None)

    nc = bacc.Bacc(None)
    x1e = nc.declare_dram_parameter("x1", [BPC, C, HW], BF16, isOutput=False)
    x2e = nc.declare_dram_parameter("x2", [BPC, C, HW], BF16, isOutput=False)
    w1e = nc.declare_dram_parameter("w1", [C, O], BF16, isOutput=False)
    w2e = nc.declare_dram_parameter("w2", [C, O], BF16, isOutput=False)
    iee = nc.declare_dram_parameter("ie", [256, 256], BF16, isOutput=False)
    ife = nc.declare_dram_parameter("if", [256, 256], BF16, isOutput=False)
    ide = nc.declare_dram_parameter("id", [H, H], BF16, isOutput=False)
    OD = BF16 if cfg.get("obf16") else F32
    oute = nc.declare_dram_parameter("out", [BPC, O, HW], OD, isOutput=True)

    ENG = {"sync": "sync", "scalar": "scalar", "gpsimd": "gpsimd", "vector": "vector"}
    xq = getattr(nc, ENG[x_eng])
    oq = getattr(nc, ENG[o_eng])

    with TileContext(nc) as tc:
        with (
            tc.tile_pool(name="wpool", bufs=1) as wpool,
            tc.tile_pool(name="xpool", bufs=xbufs) as xpool,
            tc.tile_pool(name="fpool", bufs=fbufs) as fpool,
            tc.tile_pool(name="opool", bufs=obufs) as opool,
            tc.tile_pool(name="psf", bufs=psf_bufs, space="PSUM") as psf,
            tc.tile_pool(name="pse", bufs=pse_bufs, space="PSUM") as pse,
            tc.tile_pool(name="psq", bufs=psq_bufs, space="PSUM") as psq,
            tc.tile_pool(name="psd", bufs=psd_bufs, space="PSUM") as psd,
        ):
            # ---- PE warm-up: ramp the tensor clock while DMAs land ----
            if warm:
                wa = wpool.tile([128, warm_ap], BF16, tag="warm_a", name="warm_a")
                getattr(nc, ENG[warm_eng]).memset(wa[:], 0.0)
                for wi in range(warm):
                    pw_ = psf.tile([128, warm_ap], F32, tag="psf", name=f"warm{wi}")
                    nc.tensor.matmul(
                        pw_[:], wa[:, 0:128], wa[:], start=True, stop=True
                    )

            def load_x(t, b, nsl, j_only=None, eng=None, ccsplit=False):
                pw = nsl.stop - nsl.start
                xr = {}
                for j, xe in ((1, x1e), (2, x2e)):
                    if j_only is not None and j != j_only:
                        continue
                    xt = xpool.tile([128, 4 * pw], BF16, tag=f"x{j}", name=f"x{j}_{t}")
                    if ccsplit:
                        k = 4 // int(ccsplit)
                        for g in range(int(ccsplit)):
                            c0, c1 = g * k, (g + 1) * k
                            (eng or xq).dma_start(
                                out=xt[:, c0 * pw : c1 * pw].rearrange(
                                    "p (c n) -> p c n", c=k
                                ),
                                in_=xe[b, c0 * 128 : c1 * 128, nsl].rearrange(
                                    "(c p) n -> p c n", c=k
                                ),
                            )
                    else:
                        (eng or xq).dma_start(
                            out=xt[:].rearrange("p (c n) -> p c n", c=4),
                            in_=xe[b, :, nsl].rearrange("(c p) n -> p c n", c=4),
                        )
                    xr[j] = xt
                return xr

            # ---- weights (already bf16 in DRAM; plain loads) ----
            w1r, w2r, iet, ift, idt = [], [], [], [], []
            specs = {
                "w1r": (w1r, w1e, O, 4),
                "w2r": (w2r, w2e, O, 4),
                "ie": (iet, iee, 256, 2),
                "if": (ift, ife, 256, 2),
                "id": (idt, ide, H, 4),
            }

            wsplit = cfg.get("wsplit", 4)  # load w1r/w2r in this many col pieces

            def make_w(nm):
                # one wide [128, 4*O] tile; chunk cc at free offset cc*O
                lst, ext, shp, nch = specs[nm]
                big = wpool.tile([128, nch * shp], BF16, tag=nm, name=nm)
                for cc in range(nch):
                    lst.append(big[:, cc * shp : (cc + 1) * shp])
                return big

            def load_w_piece(nm, s, ws):
                lst, ext, shp, nch = specs[nm]
                big = _wbig[nm]
                csl = slice(s * shp // ws, (s + 1) * shp // ws)
                nc.sync.dma_start(
                    out=big[:].rearrange("p (c n) -> p c n", c=nch)[:, :, csl],
                    in_=ext[:, csl].rearrange("(c p) n -> p c n", c=nch),
                )

            def load_w(nm):
                lst, ext, shp, nch = specs[nm]
                for cc in range(nch):
                    t = wpool.tile([128, shp], BF16, tag=f"{nm}{cc}", name=f"{nm}{cc}")
                    nc.sync.dma_start(out=t[:], in_=ext[cc * 128 : (cc + 1) * 128])
                    lst.append(t)

            _wbig = {"w1r": make_w("w1r"), "w2r": make_w("w2r")}
            # head order: x1(t0) -> first w1 pieces -> x2(t0) -> rest of w1 ->
            # w2 -> inverse weights, so the j=1 forward starts as early as
            # possible and each piece lands just ahead of its fc groups
            _xr_pre = {}
            _jp = cfg.get("job_pws")
            _pw0 = _jp[0][0] if _jp else PT
            _x0q = nc.gpsimd if cfg.get("x0_gpsimd") else nc.sync
            if x_first == "w0":
                # w1 piece 0 first (small; first fc group needs it), then x1,
                # then the rest: fc groups consume pieces slower than they land
                load_w_piece("w1r", 0, wsplit)
                if wsplit >= 8:
                    load_w_piece("w1r", 1, wsplit)
                _xr_pre[0] = load_x(0, 0, slice(0, _pw0), j_only=1, eng=_x0q)
                for s in range(2 if wsplit >= 8 else 1, wsplit):
                    load_w_piece("w1r", s, wsplit)
                _xr_pre[0].update(load_x(0, 0, slice(0, _pw0), j_only=2, eng=_x0q))
                for s in range(wsplit):
                    load_w_piece("w2r", s, wsplit)
            elif x_first:
                _xr_pre[0] = load_x(
                    0, 0, slice(0, _pw0), j_only=1, eng=_x0q, ccsplit=xsplit0
                )
                for s in range(min(2, wsplit)):
                    load_w_piece("w1r", s, wsplit)
                _xr_pre[0].update(load_x(0, 0, slice(0, _pw0), j_only=2, eng=_x0q))
                for s in range(min(2, wsplit), wsplit):
                    load_w_piece("w1r", s, wsplit)
                for s in range(wsplit):
                    load_w_piece("w2r", s, wsplit)
            else:
                for s in range(wsplit):
                    load_w_piece("w1r", s, wsplit)
                for s in range(wsplit):
                    load_w_piece("w2r", s, wsplit)
            load_w("ie")
            load_w("if")
            load_w("id")

            # ---- main loop over position tiles ----
            job_pws = cfg.get("job_pws")
            if job_pws:
                jobs = []
                for b in range(BPC):
                    n0 = 0
                    for pw in job_pws[b]:
                        jobs.append((len(jobs), b, n0, pw))
                        n0 += pw
                    assert n0 == HW
            else:
                jobs = [(t, (t // 2), (t % 2) * PT, PT) for t in range(nt_override)]
            if tail_split and nt_override == NT:
                ts = int(tail_split)
                lt, lb, ln0, _ = jobs.pop()
                for s in range(ts):
                    jobs.append((lt + s, lb, ln0 + s * PT // ts, PT // ts))
            dfirst = cfg.get("dfirst", False)
            fc_order = [4, 5, 6, 7, 0, 1, 2, 3] if dfirst else list(range(8))
            pair_d = [(4, 6), (5, 7)]
            pair_ef = [(0, 1), (2, 3)]

            last_t = jobs[-1][0]
            ueng_last = cfg.get("ueng_last", ueng)
            per_tile = cfg.get("per_tile", {})
            defer_store = cfg.get("defer_store", False)
            pending_store = []

            def flush_store():
                while pending_store:
                    ob, onsl, ot, opw = pending_store.pop(0)
                    oq.dma_start(
                        out=oute[ob].rearrange("(c q p) n -> p c q n", c=2, q=4)[
                            :, :, :, onsl
                        ],
                        in_=ot[:].rearrange("p (c q n) -> p c q n", c=2, q=4)[
                            :, :, :, 0:opw
                        ],
                    )

            for t, b, n0, pw in jobs:
                if defer_store:
                    flush_store()
                nsl = slice(n0, n0 + pw)
                xr = _xr_pre[t] if t in _xr_pre else load_x(t, b, nsl)
                W_ = slice(0, pw)
                ue = ueng_last if t == last_t else ueng
                ptc = per_tile.get(t) or per_tile.get(str(t)) or {}
                ue = ptc.get("ueng", ue)
                fft = {}
                prod = {}
                cch = []
                cs = [None] * 4

                def fwd(j, fcs, t=t, xr=xr, pw=pw, W_=W_, fft=fft):
                    wr = w1r if j == 1 else w2r
                    for fc in fcs:
                        ps = psf.tile([128, pw], F32, tag="psf", name=f"psf{j}_{fc}_{t}")
                        for cc in range(4):
                            nc.tensor.matmul(
                                ps[:, W_],
                                wr[cc][:, fc * 128 : (fc + 1) * 128],
                                xr[j][:, cc * pw : (cc + 1) * pw],
                                start=(cc == 0),
                                stop=(cc == 3),
                            )
                        ft = fpool.tile(
                            [128, pw], BF16, tag=f"fft{j}_{fc}", name=f"fft{j}_{fc}_{t}"
                        )
                        nc.scalar.copy(out=ft[:, W_], in_=ps[:, W_])
                        fft[(j, fc)] = ft

                cm_last = cfg.get("cmult_last", "dddd")

                def cmult(pairs, t=t, W_=W_, fft=fft, prod=prod, cm_last=cm_last):
                    # complex multiply (bf16, all-SBUF): chunk pairs (re,im)
                    for re_c, im_c in pairs:
                        pidx = {0: 0, 2: 1, 4: 2, 5: 3}[re_c]
                        eng = (
                            nc.gpsimd
                            if t == last_t and cm_last[pidx] == "p"
                            else nc.vector
                        )
                        a1, b1 = fft[(1, re_c)], fft[(1, im_c)]
                        a2, b2 = fft[(2, re_c)], fft[(2, im_c)]
                        m1 = fpool.tile([128, pw], BF16, tag="m1", name=f"m1_{re_c}_{t}")
                        m2 = fpool.tile([128, pw], BF16, tag="m2", name=f"m2_{re_c}_{t}")
                        pr = fpool.tile(
                            [128, pw], BF16, tag=f"pr{re_c}", name=f"pr{re_c}_{t}"
                        )
                        pi = fpool.tile(
                            [128, pw], BF16, tag=f"pi{im_c}", name=f"pi{im_c}_{t}"
                        )
                        if re_c in cfg.get("cm_split", ()) and t != last_t:
                            # pr-chain on `eng`, independent pi-chain on Pool
                            m3 = fpool.tile(
                                [128, pw], BF16, tag="m3", name=f"m3_{re_c}_{t}"
                            )
                            m4 = fpool.tile(
                                [128, pw], BF16, tag="m4", name=f"m4_{re_c}_{t}"
                            )
                            eng.tensor_mul(m1[:, W_], a1[:, W_], a2[:, W_])
                            eng.tensor_mul(m2[:, W_], b1[:, W_], b2[:, W_])
                            eng.tensor_sub(pr[:, W_], m1[:, W_], m2[:, W_])
                            nc.gpsimd.tensor_mul(m3[:, W_], a1[:, W_], b2[:, W_])
                            nc.gpsimd.tensor_mul(m4[:, W_], b1[:, W_], a2[:, W_])
                            nc.gpsimd.tensor_add(pi[:, W_], m3[:, W_], m4[:, W_])
                        else:
                            eng.tensor_mul(m1[:, W_], a1[:, W_], a2[:, W_])
                            eng.tensor_mul(m2[:, W_], b1[:, W_], b2[:, W_])
                            eng.tensor_sub(pr[:, W_], m1[:, W_], m2[:, W_])
                            eng.tensor_mul(m1[:, W_], a1[:, W_], b2[:, W_])
                            eng.tensor_mul(m2[:, W_], b1[:, W_], a2[:, W_])
                            eng.tensor_add(pi[:, W_], m1[:, W_], m2[:, W_])
                        if re_c == 0:
                            # row 0 of the (0,1) pair: DC_e (re) and Nyquist-256
                            # (held in im slot row 0) are real-only products
                            feng = (
                                nc.gpsimd if cfg.get("fix_eng") == "p" else eng
                            )
                            feng.tensor_mul(pr[0:1, W_], a1[0:1, W_], a2[0:1, W_])
                            feng.tensor_mul(pi[0:1, W_], b1[0:1, W_], b2[0:1, W_])
                        prod[re_c] = pr
                        prod[im_c] = pi

                act_stage = cfg.get("act_stage", False) or (
                    t == last_t and cfg.get("act_stage_last", False)
                )
                act_f_t = act_f if act_f is not None else act_stage
                act_d_t = act_d if act_d is not None else act_stage
                if t == last_t and cfg.get("act_stage_last", False):
                    act_f_t = act_d_t = True
                act_f_t = ptc.get("act_f", act_f_t)
                act_d_t = ptc.get("act_d", act_d_t)

                def inv_ef(t=t, W_=W_, prod=prod, cch=cch, cs=cs, ue=ue,
                           act_f_t=act_f_t):
                    # inverse level2: e,f [256] then c = unfold2(e,f) in SBUF
                    for oc2 in range(2):
                        osl2 = slice(oc2 * 128, (oc2 + 1) * 128)
                        pe_ = pse.tile([128, pw], F32, tag="pse", name=f"pse{oc2}_{t}")
                        pf_ = psq.tile([128, pw], F32, tag="psq", name=f"psq{oc2}_{t}")
                        for rc in range(2):
                            nc.tensor.matmul(
                                pe_[:, W_], iet[rc][:, osl2], prod[rc][:, W_],
                                start=(rc == 0), stop=(rc == 1),
                            )
                        for rc in range(2):
                            nc.tensor.matmul(
                                pf_[:, W_], ift[rc][:, osl2], prod[2 + rc][:, W_],
                                start=(rc == 0), stop=(rc == 1),
                            )
                        es = opool.tile(
                            [128, pw], OD, tag=f"es{oc2}", name=f"es{oc2}_{t}"
                        )
                        nc.scalar.copy(out=es[:, W_], in_=pe_[:, W_])
                        if act_f_t:
                            # drain pf_ to SBUF too so unfolds are SBUF-only
                            # (GPSIMD cannot touch PSUM) and psq frees fast
                            fs = opool.tile(
                                [128, pw], OD, tag=f"fs{oc2}", name=f"fs{oc2}_{t}"
                            )
                            nc.scalar.copy(out=fs[:, W_], in_=pf_[:, W_])
                            cch.append((es, fs))
                        else:
                            cch.append((es, pf_))
                    for oc in range(4):
                        es, pf_ = cch[oc % 2]
                        ct = opool.tile([128, pw], OD, tag=f"c{oc}", name=f"c{oc}_{t}")
                        ceng = nc.gpsimd if ue[oc] == "p" else nc.vector
                        if oc < 2:
                            ceng.tensor_add(ct[:, W_], es[:, W_], pf_[:, W_])
                        else:
                            ceng.tensor_sub(ct[:, W_], es[:, W_], pf_[:, W_])
                        cs[oc] = ct

                pds = [None] * 4

                def inv_d(ocs, rc_order=(0, 1, 2, 3), t=t, W_=W_, prod=prod, pds=pds,
                          act_d_t=act_d_t):
                    spread = ptc.get("spread_dbanks", cfg.get("spread_dbanks", False))
                    for oc in ocs:
                        osl = slice(oc * 128, (oc + 1) * 128)
                        if spread and oc == 2:
                            pd = psq.tile([128, pw], F32, tag="psq", name=f"psd{oc}_{t}")
                        elif spread and oc == 3:
                            pd = pse.tile([128, pw], F32, tag="pse", name=f"psd{oc}_{t}")
                        else:
                            pd = psd.tile([128, pw], F32, tag="psd", name=f"psd{oc}_{t}")
                        for i, rc in enumerate(rc_order):
                            nc.tensor.matmul(
                                pd[:, W_], idt[rc][:, osl], prod[4 + rc][:, W_],
                                start=(i == 0), stop=(i == 3),
                            )
                        if act_d_t:
                            ds = opool.tile(
                                [128, pw], OD, tag=f"ds{oc}", name=f"ds{oc}_{t}"
                            )
                            nc.scalar.copy(out=ds[:, W_], in_=pd[:, W_])
                            pds[oc] = ds
                        else:
                            pds[oc] = pd

                otile = [None]

                def unfold(ocs, t=t, b=b, nsl=nsl, W_=W_, cs=cs, pds=pds, ue=ue,
                           pw=pw, n0=n0, otile=otile):
                    for oc in ocs:
                        osl = slice(oc * 128, (oc + 1) * 128)
                        leng = nc.gpsimd if ue[4 + 2 * oc] == "p" else nc.vector
                        heng = nc.gpsimd if ue[5 + 2 * oc] == "p" else nc.vector
                        if t == last_t and cfg.get("packed_oc_last") == "oc":
                            lohi = opool.tile(
                                [128, 2 * pw], OD, tag=f"lh{oc}", name=f"lh{oc}_{t}"
                            )
                            lo = lohi[:, 0:pw]
                            hi = lohi[:, pw : 2 * pw]
                            leng.tensor_add(lo, cs[oc][:, W_], pds[oc][:, W_])
                            heng.tensor_sub(hi, cs[oc][:, W_], pds[oc][:, W_])
                            sq = (
                                getattr(nc, ENG[store_eng_last[oc]])
                                if store_eng_last
                                else oq
                            )
                            sq.dma_start(
                                out=oute[b].rearrange(
                                    "(c q p) n -> q p c n", c=2, q=4
                                )[oc][:, :, nsl],
                                in_=lohi[:].rearrange("p (c n) -> p c n", c=2),
                            )
                            continue
                        if packed_oc == "full" and not (t == last_t and store_eng_last):
                            if otile[0] is None:
                                otile[0] = opool.tile(
                                    [128, 8 * pw], OD, tag="ot", name=f"ot_{t}"
                                )
                            ot = otile[0]
                            lo = ot[:, oc * pw : (oc + 1) * pw]
                            hi = ot[:, (4 + oc) * pw : (5 + oc) * pw]
                            leng.tensor_add(lo, cs[oc][:, W_], pds[oc][:, W_])
                            heng.tensor_sub(hi, cs[oc][:, W_], pds[oc][:, W_])
                            if oc == 3:
                                if defer_store:
                                    pending_store.append((b, nsl, ot, pw))
                                else:
                                    oq.dma_start(
                                        out=oute[b].rearrange(
                                            "(c q p) n -> p c q n", c=2, q=4
                                        )[:, :, :, nsl],
                                        in_=ot[:].rearrange(
                                            "p (c q n) -> p c q n", c=2, q=4
                                        )[:, :, :, 0:pw],
                                    )
                            continue
                        if packed_oc and not (t == last_t and store_eng_last):
                            lohi = opool.tile(
                                [128, 2 * pw], OD, tag=f"lh{oc}", name=f"lh{oc}_{t}"
                            )
                            lo = lohi[:, 0:pw]
                            hi = lohi[:, pw : 2 * pw]
                            leng.tensor_add(lo, cs[oc][:, W_], pds[oc][:, W_])
                            heng.tensor_sub(hi, cs[oc][:, W_], pds[oc][:, W_])
                            oq.dma_start(
                                out=oute[b].rearrange(
                                    "(c q p) n -> q p c n", c=2, q=4
                                )[oc][:, :, nsl],
                                in_=lohi[:].rearrange("p (c n) -> p c n", c=2)[
                                    :, :, 0:pw
                                ],
                            )
                        else:
                            lot = opool.tile(
                                [128, pw], OD, tag=f"lo{oc}", name=f"lo{oc}_{t}"
                            )
                            hit = opool.tile(
                                [128, pw], OD, tag=f"hi{oc}", name=f"hi{oc}_{t}"
                            )
                            lo, hi = lot[:, W_], hit[:, W_]
                            leng.tensor_add(lo, cs[oc][:, W_], pds[oc][:, W_])
                            heng.tensor_sub(hi, cs[oc][:, W_], pds[oc][:, W_])
                            if t == last_t and store_eng_last:
                                lq = getattr(nc, ENG[store_eng_last[2 * oc]])
                                hq = getattr(nc, ENG[store_eng_last[2 * oc + 1]])
                            else:
                                lq, hq = oq, oq
                            lq.dma_start(out=oute[b, osl, nsl], in_=lo)
                            hq.dma_start(
                                out=oute[
                                    b, slice(512 + oc * 128, 512 + (oc + 1) * 128), nsl
                                ],
                                in_=hi,
                            )

                ilv = cfg.get("ilv", "j2")
                if t == last_t and cfg.get("ilv_last") is not None:
                    ilv = cfg.get("ilv_last")
                if ilv:
                    # emit each complex multiply as soon as its pair of chunks
                    # is transformed, so the tail inverse never waits on a
                    # long cmult chain
                    emit = {1: [(0, 1)], 3: [(2, 3)], 6: [(4, 6)], 7: [(5, 7)]}
                    if ilv == "j2":
                        fwd(1, [0, 1, 2, 3, 4, 6, 5, 7])
                        for fc in [0, 1, 2, 3, 4, 6, 5, 7]:
                            fwd(2, [fc])
                            if fc in emit:
                                cmult(emit[fc])
                    else:
                        for fc in [0, 1, 2, 3, 4, 6, 5, 7]:
                            fwd(1, [fc])
                            fwd(2, [fc])
                            if fc in emit:
                                cmult(emit[fc])
                    inv_ef()
                    for oc in range(4):
                        inv_d([oc], rc_order=(0, 2, 1, 3))
                        unfold([oc])
                elif dfirst:
                    fwd(1, fc_order)
                    fwd(2, fc_order)
                    cmult(pair_d)
                    inv_d([0, 1, 2, 3])
                    cmult(pair_ef)
                    inv_ef()
                    unfold([0, 1, 2, 3])
                else:
                    fwd(1, fc_order)
                    fwd(2, fc_order)
                    cmult(pair_ef + pair_d)
                    inv_ef()
                    for oc in range(4):
                        inv_d([oc])
                        unfold([oc])
            if defer_store:
                flush_store()

    nc.finalize()
    return nc


_NC_CACHE = None
_NC_CFG = None
KCFG = {
    "o_eng": "gpsimd",
    "x_eng": "sync",
    "psf_bufs": 3,
    "psd_bufs": 3,
    "ueng": "ppdddddddddd",
    "packed_oc": "full",
    "obf16": True,
    "wsplit": 4,
    "act_f": True,
    "act_d": False,
    "ilv": False,
    "ilv_last": "j2",
    "act_stage_last": False,
    "ueng_last": "dddddddddddd",
    "cmult_last": "dddd",
    "store_eng_last": ["sync", "scalar", "sync", "scalar"],
    "packed_oc_last": "oc",
    "per_tile": {"7": {"ueng": "ppppdddddddd", "act_d": True}},
    "fix_eng": "p",
}


def _make_in_maps(x1, x2, sketch1, sketch2):
    w1, w2, ie, if_, idm = _build_host_matrices(sketch1, sketch2)
    x1f = np.asarray(x1, dtype=np.float32).reshape(B, C, HW).astype(BF16NP)
    x2f = np.asarray(x2, dtype=np.float32).reshape(B, C, HW).astype(BF16NP)
    in_maps = []
    for i in range(NCORES):
        bs = slice(i * BPC, (i + 1) * BPC)
        in_maps.append(
            {
                "x1": np.ascontiguousarray(x1f[bs]),
                "x2": np.ascontiguousarray(x2f[bs]),
                "w1": w1,
                "w2": w2,
                "ie": ie,
                "if": if_,
                "id": idm,
            }
        )
    return in_maps


def kernel(x1, x2, sketch1, sketch2):
    global _NC_CACHE, _NC_CFG
    if _NC_CACHE is None or _NC_CFG != KCFG:
        _NC_CACHE = _build_program(KCFG)
        _NC_CFG = dict(KCFG)
    nc = _NC_CACHE
    in_maps = _make_in_maps(x1, x2, sketch1, sketch2)
    res = run_bass_kernel_spmd(nc, in_maps, list(range(NCORES)))
    out = np.concatenate([res.results[i]["out"] for i in range(NCORES)], axis=0)
    return out.reshape(B, O, 28, 28).astype(np.float32)


if __name__ == "__main__":
    rng = np.random.default_rng(0)
    x1 = rng.standard_normal((B, C, 28, 28)).astype(np.float32)
    x2 = rng.standard_normal((B, C, 28, 28)).astype(np.float32)
    h1 = rng.integers(0, O, C)
    s1 = rng.integers(0, 2, C) * 2.0 - 1.0
    h2 = rng.integers(0, O, C)
    s2 = rng.integers(0, 2, C) * 2.0 - 1.0
    sk1 = np.zeros((O, C), np.float32)
    sk1[h1, np.arange(C)] = s1
    sk2 = np.zeros((O, C), np.float32)
    sk2[h2, np.arange(C)] = s2
    got = kernel(x1, x2, sk1, sk2)
    p1 = np.einsum("bchw,oc->bohw", x1, sk1).reshape(B, O, HW)
    p2 = np.einsum("bchw,oc->bohw", x2, sk2).reshape(B, O, HW)
    ref = np.fft.ifft(np.fft.fft(p1, axis=1) * np.fft.fft(p2, axis=1), axis=1).real
    err = np.abs(got.reshape(B, O, HW) - ref).max() / np.abs(ref).max()
    print("self-test max rel err:", err)



# revision 30
# speedup vs baseline: 1.0006x; 1.0006x over previous
"""Compact bilinear pooling kernel for 8 Trainium2 NeuronCores.

Algorithm (host side folds everything into matmul weights):
  out[b,:,n] = circconv_1024(S1 @ x1[b,:,n], S2 @ x2[b,:,n])
Decomposed via x^1024-1 = (x^512-1)(x^512+1):
  cyclic-512 branch (rFFT512) + negacyclic-512 branch (odd DFT), both fused
  with the count-sketch matrices into dense real forward matrices
  W_j [512c -> 1024 freq rows], applied as bf16 matmuls. Middle (complex
  multiply) runs on bf16 SBUF tiles on the vector engine. Inverse transforms
  are two block-diagonal [512 rows -> 512 outs] bf16 matmuls; the final
  unfold (c+d, c-d) is split between the vector and gpsimd engines reading
  inverse PSUM and writing packed bf16 output tiles per position tile.

v3 layout (current KCFG):
  - x1/x2 and all weights cast to bf16 on the HOST; x/weight loads are plain
    HWDGE transfers on the sync (SP) queue.
  - output staged in bf16 (host upcasts to f32 after gather; ~0.1% extra
    quantization, well inside the 2e-2 gate): halves store traffic and makes
    staged unfold ops pure-bf16 (2x DVE rate).
  - steady tiles: all 8 lo/hi outputs packed into ONE [128, 8*PT] bf16 tile,
    stored as a single SWDGE DMA on the Pool queue (25ns SEQ issue; descgen
    on the idle Pool engine) -> HWDGE/SP freed, DMA count per tile = 3.
  - unfold engine split: cs0/cs1 on Pool, rest on DVE; the 2 single-row
    DC/Nyquist cmult fixups on Pool (each costs a full row-time).
  - last tile: cs on Pool, act_d staging (Act drains psd -> no PE stall on
    PSUM banks; unfolds become 265ns bf16 ops), stores packed per-oc and
    spread across sync/scalar HWDGE queues to shorten the tail chain.
  - PE warm-up matmuls pin pe_busy_start early so real matmuls run at full
    p-state; head is DMA-pipeline-bound (x1 + first w piece ~4.7us).

Sharding: batch 32 -> 4 per core (data parallel), weights replicated.
Layout: channels/freq rows on SBUF partitions, positions on free axis.
No transposes anywhere.
"""
import sys

sys.path.insert(0, "/opt/trn_rl_repo")

import numpy as np
import ml_dtypes
import concourse.bass as bass
import concourse.mybir as mybir
from concourse import bacc
from concourse.tile import TileContext
from concourse.bass_utils import run_bass_kernel_spmd

B, C, HW, O = 32, 512, 784, 1024
NCORES = 8
BPC = B // NCORES  # 4 batches per core
PT = 392  # positions per tile (784 = 2*392; tiles never cross batch bounds)
NT = BPC * HW // PT  # 8 pos tiles per core
H = O // 2  # 512
F32, F32R, BF16 = mybir.dt.float32, mybir.dt.float32r, mybir.dt.bfloat16
BF16NP = ml_dtypes.bfloat16


def _build_host_matrices(sketch1, sketch2):
    """Fused fwd [512 c, 1024 freq-rows]; inverse IE/IF [256,256], ID [512,512].

    Level-2 folded row layout: e=rfft256, f=oddDFT256, d=oddDFT512; inverse
    weights carry the unfold 1/2 factors. All returned as bf16.
    """

    def build_fwd(sketch):
        sk = np.asarray(sketch, dtype=np.float64)
        Sp = sk[:H] + sk[H:]
        Sm = sk[:H] - sk[H:]
        Spp = Sp[:256] + Sp[256:]
        Spm = Sp[:256] - Sp[256:]
        n2 = np.arange(256)[None, :]
        k2 = np.arange(129)[:, None]
        Mc2 = np.exp(-2j * np.pi * k2 * n2 / 256) @ Spp
        k2f = np.arange(128)[:, None]
        Mo2 = np.exp(-2j * np.pi * n2 * (2 * k2f + 1) / 512) @ Spm
        n = np.arange(H)[None, :]
        ko = np.arange(256)[:, None]
        Mo = np.exp(-2j * np.pi * n * (2 * ko + 1) / O) @ Sm
        W = np.zeros((O, C))
        W[0:128] = Mc2[0:128].real
        W[128] = Mc2[128].real
        W[129:256] = Mc2[1:128].imag
        W[256:384] = Mo2.real
        W[384:512] = Mo2.imag
        W[512:768] = Mo.real
        W[768:1024] = Mo.imag
        return np.ascontiguousarray(W.T).astype(BF16NP)  # [C, O]

    j2 = np.arange(256)[None, :]
    k = np.arange(128)[:, None]
    IE = np.zeros((256, 256))
    IE[0:128] = 2 * np.cos(2 * np.pi * k * j2 / 256) / 256
    IE[0] = 1.0 / 256
    IE[128] = np.cos(np.pi * j2) / 256
    ki = np.arange(1, 128)[:, None]
    IE[129:256] = -2 * np.sin(2 * np.pi * ki * j2 / 256) / 256
    IF = np.zeros((256, 256))
    IF[0:128] = 2 * np.cos(2 * np.pi * (2 * k + 1) * j2 / 512) / 256
    IF[128:256] = -2 * np.sin(2 * np.pi * (2 * k + 1) * j2 / 512) / 256
    j = np.arange(H)[None, :]
    ko = np.arange(256)[:, None]
    ID = np.zeros((H, H))
    ID[0:256] = 2 * np.cos(2 * np.pi * (2 * ko + 1) * j / O) / H
    ID[256:512] = -2 * np.sin(2 * np.pi * (2 * ko + 1) * j / O) / H
    return (
        build_fwd(sketch1),
        build_fwd(sketch2),
        (IE / 4).astype(BF16NP),
        (IF / 4).astype(BF16NP),
        (ID / 2).astype(BF16NP),
    )


def _build_program(cfg=None):
    cfg = cfg or {}
    psf_bufs = cfg.get("psf_bufs", 2)
    pse_bufs = cfg.get("pse_bufs", 1)
    psq_bufs = cfg.get("psq_bufs", 1)
    psd_bufs = cfg.get("psd_bufs", 4)
    xbufs = cfg.get("xbufs", 2)
    fbufs = cfg.get("fbufs", 3)
    obufs = cfg.get("obufs", 2)
    # engine per unfold op [cs0..cs3, lo0,hi0,lo1,hi1,lo2,hi2,lo3,hi3]
    ueng = cfg.get("ueng", "ddpp" + "dp" * 4)
    store_split = cfg.get("store_split", 4)  # 1 or 4 pieces per tile
    warm = cfg.get("warm", 6)  # warm-up matmuls to ramp PE clock
    warm_ap = cfg.get("warm_ap", 392)
    x_eng = cfg.get("x_eng", "sync")  # HWDGE queue for x loads
    o_eng = cfg.get("o_eng", "scalar")  # queue for output stores
    es_skip = cfg.get("es_skip", False)  # cs reads pe_ PSUM directly
    tail_split = cfg.get("tail_split", False)
    x_first = cfg.get("x_first", True)  # lead DMA queues with tile-0 x loads
    nt_override = cfg.get("nt", NT)
    packed_oc = cfg.get("packed_oc", False)  # lo+hi in one tile, 1 DMA per oc
    act_f = cfg.get("act_f", None)  # drain pf_ PSUM->SBUF (needed for cs on pool)
    act_d = cfg.get("act_d", None)  # drain pd PSUM->SBUF (needed for lo/hi on pool)
    xsplit0 = cfg.get("xsplit0", False)  # split tile-0 x1 load per cc chunk
    warm_eng = cfg.get("warm_eng", "gpsimd")  # engine for warm tile memset
    # per-store queue pattern for the last tile (when not packed)
    store_eng_last = cfg.get("store_eng_last", None)

    nc = bacc.Bacc(None)
    x1e = nc.declare_dram_parameter("x1", [BPC, C, HW], BF16, isOutput=False)
    x2e = nc.declare_dram_parameter("x2", [BPC, C, HW], BF16, isOutput=False)
    w1e = nc.declare_dram_parameter("w1", [C, O], BF16, isOutput=False)
    w2e = nc.declare_dram_parameter("w2", [C, O], BF16, isOutput=False)
    iee = nc.declare_dram_parameter("ie", [256, 256], BF16, isOutput=False)
    ife = nc.declare_dram_parameter("if", [256, 256], BF16, isOutput=False)
    ide = nc.declare_dram_parameter("id", [H, H], BF16, isOutput=False)
    OD = BF16 if cfg.get("obf16") else F32
    oute = nc.declare_dram_parameter("out", [BPC, O, HW], OD, isOutput=True)

    ENG = {"sync": "sync", "scalar": "scalar", "gpsimd": "gpsimd", "vector": "vector"}
    xq = getattr(nc, ENG[x_eng])
    oq = getattr(nc, ENG[o_eng])

    with TileContext(nc) as tc:
        with (
            tc.tile_pool(name="wpool", bufs=1) as wpool,
            tc.tile_pool(name="xpool", bufs=xbufs) as xpool,
            tc.tile_pool(name="fpool", bufs=fbufs) as fpool,
            tc.tile_pool(name="opool", bufs=obufs) as opool,
            tc.tile_pool(name="psf", bufs=psf_bufs, space="PSUM") as psf,
            tc.tile_pool(name="pse", bufs=pse_bufs, space="PSUM") as pse,
            tc.tile_pool(name="psq", bufs=psq_bufs, space="PSUM") as psq,
            tc.tile_pool(name="psd", bufs=psd_bufs, space="PSUM") as psd,
        ):
            # ---- PE warm-up: ramp the tensor clock while DMAs land ----
            if warm:
                wa = wpool.tile([128, warm_ap], BF16, tag="warm_a", name="warm_a")
                getattr(nc, ENG[warm_eng]).memset(wa[:], 0.0)
                for wi in range(warm):
                    pw_ = psf.tile([128, warm_ap], F32, tag="psf", name=f"warm{wi}")
                    nc.tensor.matmul(
                        pw_[:], wa[:, 0:128], wa[:], start=True, stop=True
                    )

            def load_x(t, b, nsl, j_only=None, eng=None, ccsplit=False):
                pw = nsl.stop - nsl.start
                xr = {}
                for j, xe in ((1, x1e), (2, x2e)):
                    if j_only is not None and j != j_only:
                        continue
                    xt = xpool.tile([128, 4 * pw], BF16, tag=f"x{j}", name=f"x{j}_{t}")
                    if ccsplit:
                        k = 4 // int(ccsplit)
                        for g in range(int(ccsplit)):
                            c0, c1 = g * k, (g + 1) * k
                            (eng or xq).dma_start(
                                out=xt[:, c0 * pw : c1 * pw].rearrange(
                                    "p (c n) -> p c n", c=k
                                ),
                                in_=xe[b, c0 * 128 : c1 * 128, nsl].rearrange(
                                    "(c p) n -> p c n", c=k
                                ),
                            )
                    else:
                        (eng or xq).dma_start(
                            out=xt[:].rearrange("p (c n) -> p c n", c=4),
                            in_=xe[b, :, nsl].rearrange("(c p) n -> p c n", c=4),
                        )
                    xr[j] = xt
                return xr

            # ---- weights (already bf16 in DRAM; plain loads) ----
            w1r, w2r, iet, ift, idt = [], [], [], [], []
            specs = {
                "w1r": (w1r, w1e, O, 4),
                "w2r": (w2r, w2e, O, 4),
                "ie": (iet, iee, 256, 2),
                "if": (ift, ife, 256, 2),
                "id": (idt, ide, H, 4),
            }

            wsplit = cfg.get("wsplit", 4)  # load w1r/w2r in this many col pieces

            def make_w(nm):
                # one wide [128, 4*O] tile; chunk cc at free offset cc*O
                lst, ext, shp, nch = specs[nm]
                big = wpool.tile([128, nch * shp], BF16, tag=nm, name=nm)
                for cc in range(nch):
                    lst.append(big[:, cc * shp : (cc + 1) * shp])
                return big

            def load_w_piece(nm, s, ws):
                lst, ext, shp, nch = specs[nm]
                big = _wbig[nm]
                csl = slice(s * shp // ws, (s + 1) * shp // ws)
                nc.sync.dma_start(
                    out=big[:].rearrange("p (c n) -> p c n", c=nch)[:, :, csl],
                    in_=ext[:, csl].rearrange("(c p) n -> p c n", c=nch),
                )

            def load_w(nm):
                lst, ext, shp, nch = specs[nm]
                for cc in range(nch):
                    t = wpool.tile([128, shp], BF16, tag=f"{nm}{cc}", name=f"{nm}{cc}")
                    nc.sync.dma_start(out=t[:], in_=ext[cc * 128 : (cc + 1) * 128])
                    lst.append(t)

            _wbig = {"w1r": make_w("w1r"), "w2r": make_w("w2r")}
            # head order: x1(t0) -> first w1 pieces -> x2(t0) -> rest of w1 ->
            # w2 -> inverse weights, so the j=1 forward starts as early as
            # possible and each piece lands just ahead of its fc groups
            _xr_pre = {}
            _jp = cfg.get("job_pws")
            _pw0 = _jp[0][0] if _jp else PT
            _x0q = nc.gpsimd if cfg.get("x0_gpsimd") else nc.sync
            if x_first == "w0":
                # w1 piece 0 first (small; first fc group needs it), then x1,
                # then the rest: fc groups consume pieces slower than they land
                load_w_piece("w1r", 0, wsplit)
                if wsplit >= 8:
                    load_w_piece("w1r", 1, wsplit)
                _xr_pre[0] = load_x(0, 0, slice(0, _pw0), j_only=1, eng=_x0q)
                for s in range(2 if wsplit >= 8 else 1, wsplit):
                    load_w_piece("w1r", s, wsplit)
                _xr_pre[0].update(load_x(0, 0, slice(0, _pw0), j_only=2, eng=_x0q))
                for s in range(wsplit):
                    load_w_piece("w2r", s, wsplit)
            elif x_first:
                _xr_pre[0] = load_x(
                    0, 0, slice(0, _pw0), j_only=1, eng=_x0q, ccsplit=xsplit0
                )
                for s in range(min(2, wsplit)):
                    load_w_piece("w1r", s, wsplit)
                _xr_pre[0].update(load_x(0, 0, slice(0, _pw0), j_only=2, eng=_x0q))
                for s in range(min(2, wsplit), wsplit):
                    load_w_piece("w1r", s, wsplit)
                for s in range(wsplit):
                    load_w_piece("w2r", s, wsplit)
            else:
                for s in range(wsplit):
                    load_w_piece("w1r", s, wsplit)
                for s in range(wsplit):
                    load_w_piece("w2r", s, wsplit)
            load_w("ie")
            load_w("if")
            load_w("id")

            # ---- main loop over position tiles ----
            job_pws = cfg.get("job_pws")
            if job_pws:
                jobs = []
                for b in range(BPC):
                    n0 = 0
                    for pw in job_pws[b]:
                        jobs.append((len(jobs), b, n0, pw))
                        n0 += pw
                    assert n0 == HW
            else:
                jobs = [(t, (t // 2), (t % 2) * PT, PT) for t in range(nt_override)]
            if tail_split and nt_override == NT:
                ts = int(tail_split)
                lt, lb, ln0, _ = jobs.pop()
                for s in range(ts):
                    jobs.append((lt + s, lb, ln0 + s * PT // ts, PT // ts))
            dfirst = cfg.get("dfirst", False)
            fc_order = [4, 5, 6, 7, 0, 1, 2, 3] if dfirst else list(range(8))
            pair_d = [(4, 6), (5, 7)]
            pair_ef = [(0, 1), (2, 3)]

            last_t = jobs[-1][0]
            ueng_last = cfg.get("ueng_last", ueng)
            per_tile = cfg.get("per_tile", {})
            defer_store = cfg.get("defer_store", False)
            pending_store = []

            def flush_store():
                while pending_store:
                    ob, onsl, ot, opw = pending_store.pop(0)
                    oq.dma_start(
                        out=oute[ob].rearrange("(c q p) n -> p c q n", c=2, q=4)[
                            :, :, :, onsl
                        ],
                        in_=ot[:].rearrange("p (c q n) -> p c q n", c=2, q=4)[
                            :, :, :, 0:opw
                        ],
                    )

            for t, b, n0, pw in jobs:
                if defer_store:
                    flush_store()
                nsl = slice(n0, n0 + pw)
                xr = _xr_pre[t] if t in _xr_pre else load_x(t, b, nsl)
                W_ = slice(0, pw)
                ue = ueng_last if t == last_t else ueng
                ptc = per_tile.get(t) or per_tile.get(str(t)) or {}
                ue = ptc.get("ueng", ue)
                fft = {}
                prod = {}
                cch = []
                cs = [None] * 4

                def fwd(j, fcs, t=t, xr=xr, pw=pw, W_=W_, fft=fft):
                    wr = w1r if j == 1 else w2r
                    for fc in fcs:
                        ps = psf.tile([128, pw], F32, tag="psf", name=f"psf{j}_{fc}_{t}")
                        for cc in range(4):
                            nc.tensor.matmul(
                                ps[:, W_],
                                wr[cc][:, fc * 128 : (fc + 1) * 128],
                                xr[j][:, cc * pw : (cc + 1) * pw],
                                start=(cc == 0),
                                stop=(cc == 3),
                            )
                        ft = fpool.tile(
                            [128, pw], BF16, tag=f"fft{j}_{fc}", name=f"fft{j}_{fc}_{t}"
                        )
                        nc.scalar.copy(out=ft[:, W_], in_=ps[:, W_])
                        fft[(j, fc)] = ft

                cm_last = cfg.get("cmult_last", "dddd")

                def cmult(pairs, t=t, W_=W_, fft=fft, prod=prod, cm_last=cm_last):
                    # complex multiply (bf16, all-SBUF): chunk pairs (re,im)
                    for re_c, im_c in pairs:
                        pidx = {0: 0, 2: 1, 4: 2, 5: 3}[re_c]
                        eng = (
                            nc.gpsimd
                            if t == last_t and cm_last[pidx] == "p"
                            else nc.vector
                        )
                        a1, b1 = fft[(1, re_c)], fft[(1, im_c)]
                        a2, b2 = fft[(2, re_c)], fft[(2, im_c)]
                        m1 = fpool.tile([128, pw], BF16, tag="m1", name=f"m1_{re_c}_{t}")
                        m2 = fpool.tile([128, pw], BF16, tag="m2", name=f"m2_{re_c}_{t}")
                        pr = fpool.tile(
                            [128, pw], BF16, tag=f"pr{re_c}", name=f"pr{re_c}_{t}"
                        )
                        pi = fpool.tile(
                            [128, pw], BF16, tag=f"pi{im_c}", name=f"pi{im_c}_{t}"
                        )
                        _cms = (
                            cfg.get("cm_split_last", ())
                            if t == last_t
                            else cfg.get("cm_split", ())
                        )
                        if re_c in _cms:
                            # pr-chain on `eng`, independent pi-chain on Pool
                            m3 = fpool.tile(
                                [128, pw], BF16, tag="m3", name=f"m3_{re_c}_{t}"
                            )
                            m4 = fpool.tile(
                                [128, pw], BF16, tag="m4", name=f"m4_{re_c}_{t}"
                            )
                            eng.tensor_mul(m1[:, W_], a1[:, W_], a2[:, W_])
                            eng.tensor_mul(m2[:, W_], b1[:, W_], b2[:, W_])
                            eng.tensor_sub(pr[:, W_], m1[:, W_], m2[:, W_])
                            nc.gpsimd.tensor_mul(m3[:, W_], a1[:, W_], b2[:, W_])
                            nc.gpsimd.tensor_mul(m4[:, W_], b1[:, W_], a2[:, W_])
                            nc.gpsimd.tensor_add(pi[:, W_], m3[:, W_], m4[:, W_])
                        else:
                            eng.tensor_mul(m1[:, W_], a1[:, W_], a2[:, W_])
                            eng.tensor_mul(m2[:, W_], b1[:, W_], b2[:, W_])
                            eng.tensor_sub(pr[:, W_], m1[:, W_], m2[:, W_])
                            eng.tensor_mul(m1[:, W_], a1[:, W_], b2[:, W_])
                            eng.tensor_mul(m2[:, W_], b1[:, W_], a2[:, W_])
                            eng.tensor_add(pi[:, W_], m1[:, W_], m2[:, W_])
                        if re_c == 0:
                            # row 0 of the (0,1) pair: DC_e (re) and Nyquist-256
                            # (held in im slot row 0) are real-only products
                            feng = (
                                nc.gpsimd if cfg.get("fix_eng") == "p" else eng
                            )
                            feng.tensor_mul(pr[0:1, W_], a1[0:1, W_], a2[0:1, W_])
                            feng.tensor_mul(pi[0:1, W_], b1[0:1, W_], b2[0:1, W_])
                        prod[re_c] = pr
                        prod[im_c] = pi

                act_stage = cfg.get("act_stage", False) or (
                    t == last_t and cfg.get("act_stage_last", False)
                )
                act_f_t = act_f if act_f is not None else act_stage
                act_d_t = act_d if act_d is not None else act_stage
                if t == last_t and cfg.get("act_stage_last", False):
                    act_f_t = act_d_t = True
                act_f_t = ptc.get("act_f", act_f_t)
                act_d_t = ptc.get("act_d", act_d_t)

                def inv_ef(t=t, W_=W_, prod=prod, cch=cch, cs=cs, ue=ue,
                           act_f_t=act_f_t):
                    # inverse level2: e,f [256] then c = unfold2(e,f) in SBUF
                    for oc2 in range(2):
                        osl2 = slice(oc2 * 128, (oc2 + 1) * 128)
                        pe_ = pse.tile([128, pw], F32, tag="pse", name=f"pse{oc2}_{t}")
                        pf_ = psq.tile([128, pw], F32, tag="psq", name=f"psq{oc2}_{t}")
                        for rc in range(2):
                            nc.tensor.matmul(
                                pe_[:, W_], iet[rc][:, osl2], prod[rc][:, W_],
                                start=(rc == 0), stop=(rc == 1),
                            )
                        for rc in range(2):
                            nc.tensor.matmul(
                                pf_[:, W_], ift[rc][:, osl2], prod[2 + rc][:, W_],
                                start=(rc == 0), stop=(rc == 1),
                            )
                        es = opool.tile(
                            [128, pw], OD, tag=f"es{oc2}", name=f"es{oc2}_{t}"
                        )
                        nc.scalar.copy(out=es[:, W_], in_=pe_[:, W_])
                        if act_f_t:
                            # drain pf_ to SBUF too so unfolds are SBUF-only
                            # (GPSIMD cannot touch PSUM) and psq frees fast
                            fs = opool.tile(
                                [128, pw], OD, tag=f"fs{oc2}", name=f"fs{oc2}_{t}"
                            )
                            nc.scalar.copy(out=fs[:, W_], in_=pf_[:, W_])
                            cch.append((es, fs))
                        else:
                            cch.append((es, pf_))
                    for oc in range(4):
                        es, pf_ = cch[oc % 2]
                        ct = opool.tile([128, pw], OD, tag=f"c{oc}", name=f"c{oc}_{t}")
                        ceng = nc.gpsimd if ue[oc] == "p" else nc.vector
                        if oc < 2:
                            ceng.tensor_add(ct[:, W_], es[:, W_], pf_[:, W_])
                        else:
                            ceng.tensor_sub(ct[:, W_], es[:, W_], pf_[:, W_])
                        cs[oc] = ct

                pds = [None] * 4

                def inv_d(ocs, rc_order=(0, 1, 2, 3), t=t, W_=W_, prod=prod, pds=pds,
                          act_d_t=act_d_t):
                    spread = ptc.get("spread_dbanks", cfg.get("spread_dbanks", False))
                    for oc in ocs:
                        osl = slice(oc * 128, (oc + 1) * 128)
                        if spread and oc == 2:
                            pd = psq.tile([128, pw], F32, tag="psq", name=f"psd{oc}_{t}")
                        elif spread and oc == 3:
                            pd = pse.tile([128, pw], F32, tag="pse", name=f"psd{oc}_{t}")
                        else:
                            pd = psd.tile([128, pw], F32, tag="psd", name=f"psd{oc}_{t}")
                        for i, rc in enumerate(rc_order):
                            nc.tensor.matmul(
                                pd[:, W_], idt[rc][:, osl], prod[4 + rc][:, W_],
                                start=(i == 0), stop=(i == 3),
                            )
                        if act_d_t:
                            ds = opool.tile(
                                [128, pw], OD, tag=f"ds{oc}", name=f"ds{oc}_{t}"
                            )
                            nc.scalar.copy(out=ds[:, W_], in_=pd[:, W_])
                            pds[oc] = ds
                        else:
                            pds[oc] = pd

                otile = [None]

                def unfold(ocs, t=t, b=b, nsl=nsl, W_=W_, cs=cs, pds=pds, ue=ue,
                           pw=pw, n0=n0, otile=otile):
                    for oc in ocs:
                        osl = slice(oc * 128, (oc + 1) * 128)
                        leng = nc.gpsimd if ue[4 + 2 * oc] == "p" else nc.vector
                        heng = nc.gpsimd if ue[5 + 2 * oc] == "p" else nc.vector
                        if t == last_t and cfg.get("packed_oc_last") == "oc":
                            lohi = opool.tile(
                                [128, 2 * pw], OD, tag=f"lh{oc}", name=f"lh{oc}_{t}"
                            )
                            lo = lohi[:, 0:pw]
                            hi = lohi[:, pw : 2 * pw]
                            leng.tensor_add(lo, cs[oc][:, W_], pds[oc][:, W_])
                            heng.tensor_sub(hi, cs[oc][:, W_], pds[oc][:, W_])
                            sq = (
                                getattr(nc, ENG[store_eng_last[oc]])
                                if store_eng_last
                                else oq
                            )
                            sq.dma_start(
                                out=oute[b].rearrange(
                                    "(c q p) n -> q p c n", c=2, q=4
                                )[oc][:, :, nsl],
                                in_=lohi[:].rearrange("p (c n) -> p c n", c=2),
                            )
                            continue
                        if packed_oc == "full" and not (t == last_t and store_eng_last):
                            if otile[0] is None:
                                otile[0] = opool.tile(
                                    [128, 8 * pw], OD, tag="ot", name=f"ot_{t}"
                                )
                            ot = otile[0]
                            lo = ot[:, oc * pw : (oc + 1) * pw]
                            hi = ot[:, (4 + oc) * pw : (5 + oc) * pw]
                            leng.tensor_add(lo, cs[oc][:, W_], pds[oc][:, W_])
                            heng.tensor_sub(hi, cs[oc][:, W_], pds[oc][:, W_])
                            if oc == 3:
                                if defer_store:
                                    pending_store.append((b, nsl, ot, pw))
                                else:
                                    oq.dma_start(
                                        out=oute[b].rearrange(
                                            "(c q p) n -> p c q n", c=2, q=4
                                        )[:, :, :, nsl],
                                        in_=ot[:].rearrange(
                                            "p (c q n) -> p c q n", c=2, q=4
                                        )[:, :, :, 0:pw],
                                    )
                            continue
                        if packed_oc and not (t == last_t and store_eng_last):
                            lohi = opool.tile(
                                [128, 2 * pw], OD, tag=f"lh{oc}", name=f"lh{oc}_{t}"
                            )
                            lo = lohi[:, 0:pw]
                            hi = lohi[:, pw : 2 * pw]
                            leng.tensor_add(lo, cs[oc][:, W_], pds[oc][:, W_])
                            heng.tensor_sub(hi, cs[oc][:, W_], pds[oc][:, W_])
                            oq.dma_start(
                                out=oute[b].rearrange(
                                    "(c q p) n -> q p c n", c=2, q=4
                                )[oc][:, :, nsl],
                                in_=lohi[:].rearrange("p (c n) -> p c n", c=2)[
                                    :, :, 0:pw
                                ],
                            )
                        else:
                            lot = opool.tile(
                                [128, pw], OD, tag=f"lo{oc}", name=f"lo{oc}_{t}"
                            )
                            hit = opool.tile(
                                [128, pw], OD, tag=f"hi{oc}", name=f"hi{oc}_{t}"
                            )
                            lo, hi = lot[:, W_], hit[:, W_]
                            leng.tensor_add(lo, cs[oc][:, W_], pds[oc][:, W_])
                            heng.tensor_sub(hi, cs[oc][:, W_], pds[oc][:, W_])
                            if t == last_t and store_eng_last:
                                lq = getattr(nc, ENG[store_eng_last[2 * oc]])
                                hq = getattr(nc, ENG[store_eng_last[2 * oc + 1]])
                            else:
                                lq, hq = oq, oq
                            lq.dma_start(out=oute[b, osl, nsl], in_=lo)
                            hq.dma_start(
                                out=oute[
                                    b, slice(512 + oc * 128, 512 + (oc + 1) * 128), nsl
                                ],
                                in_=hi,
                            )

                ilv = cfg.get("ilv", "j2")
                if t == last_t and cfg.get("ilv_last") is not None:
                    ilv = cfg.get("ilv_last")
                if ilv:
                    # emit each complex multiply as soon as its pair of chunks
                    # is transformed, so the tail inverse never waits on a
                    # long cmult chain
                    emit = {1: [(0, 1)], 3: [(2, 3)], 6: [(4, 6)], 7: [(5, 7)]}
                    if ilv == "j2":
                        fwd(1, [0, 1, 2, 3, 4, 6, 5, 7])
                        for fc in [0, 1, 2, 3, 4, 6, 5, 7]:
                            fwd(2, [fc])
                            if fc in emit:
                                cmult(emit[fc])
                    else:
                        for fc in [0, 1, 2, 3, 4, 6, 5, 7]:
                            fwd(1, [fc])
                            fwd(2, [fc])
                            if fc in emit:
                                cmult(emit[fc])
                    inv_ef()
                    for oc in range(4):
                        inv_d([oc], rc_order=(0, 2, 1, 3))
                        unfold([oc])
                elif dfirst:
                    fwd(1, fc_order)
                    fwd(2, fc_order)
                    cmult(pair_d)
                    inv_d([0, 1, 2, 3])
                    cmult(pair_ef)
                    inv_ef()
                    unfold([0, 1, 2, 3])
                else:
                    fwd(1, fc_order)
                    fwd(2, fc_order)
                    cmult(pair_ef + pair_d)
                    inv_ef()
                    for oc in range(4):
                        inv_d([oc])
                        unfold([oc])
            if defer_store:
                flush_store()

    nc.finalize()
    return nc


_NC_CACHE = None
_NC_CFG = None
KCFG = {
    "o_eng": "gpsimd",
    "x_eng": "sync",
    "psf_bufs": 3,
    "psd_bufs": 3,
    "ueng": "ppdddddddddd",
    "packed_oc": "full",
    "obf16": True,
    "wsplit": 4,
    "act_f": True,
    "act_d": False,
    "ilv": False,
    "ilv_last": "j2",
    "act_stage_last": False,
    "ueng_last": "dddddddddddd",
    "cmult_last": "dddd",
    "store_eng_last": ["sync", "sync", "sync", "sync"],
    "packed_oc_last": "oc",
    "per_tile": {"7": {"ueng": "ppppdddddddd", "act_d": True}},
    "fix_eng": "p",
}


def _make_in_maps(x1, x2, sketch1, sketch2):
    w1, w2, ie, if_, idm = _build_host_matrices(sketch1, sketch2)
    x1f = np.asarray(x1, dtype=np.float32).reshape(B, C, HW).astype(BF16NP)
    x2f = np.asarray(x2, dtype=np.float32).reshape(B, C, HW).astype(BF16NP)
    in_maps = []
    for i in range(NCORES):
        bs = slice(i * BPC, (i + 1) * BPC)
        in_maps.append(
            {
                "x1": np.ascontiguousarray(x1f[bs]),
                "x2": np.ascontiguousarray(x2f[bs]),
                "w1": w1,
                "w2": w2,
                "ie": ie,
                "if": if_,
                "id": idm,
            }
        )
    return in_maps


def kernel(x1, x2, sketch1, sketch2):
    global _NC_CACHE, _NC_CFG
    if _NC_CACHE is None or _NC_CFG != KCFG:
        _NC_CACHE = _build_program(KCFG)
        _NC_CFG = dict(KCFG)
    nc = _NC_CACHE
    in_maps = _make_in_maps(x1, x2, sketch1, sketch2)
    res = run_bass_kernel_spmd(nc, in_maps, list(range(NCORES)))
    out = np.concatenate([res.results[i]["out"] for i in range(NCORES)], axis=0)
    return out.reshape(B, O, 28, 28).astype(np.float32)


if __name__ == "__main__":
    rng = np.random.default_rng(0)
    x1 = rng.standard_normal((B, C, 28, 28)).astype(np.float32)
    x2 = rng.standard_normal((B, C, 28, 28)).astype(np.float32)
    h1 = rng.integers(0, O, C)
    s1 = rng.integers(0, 2, C) * 2.0 - 1.0
    h2 = rng.integers(0, O, C)
    s2 = rng.integers(0, 2, C) * 2.0 - 1.0
    sk1 = np.zeros((O, C), np.float32)
    sk1[h1, np.arange(C)] = s1
    sk2 = np.zeros((O, C), np.float32)
    sk2[h2, np.arange(C)] = s2
    got = kernel(x1, x2, sk1, sk2)
    p1 = np.einsum("bchw,oc->bohw", x1, sk1).reshape(B, O, HW)
    p2 = np.einsum("bchw,oc->bohw", x2, sk2).reshape(B, O, HW)
    ref = np.fft.ifft(np.fft.fft(p1, axis=1) * np.fft.fft(p2, axis=1), axis=1).real
    err = np.abs(got.reshape(B, O, HW) - ref).max() / np.abs(ref).max()
    print("self-test max rel err:", err)



# revision 35
# speedup vs baseline: 1.0071x; 1.0065x over previous
"""Compact bilinear pooling kernel for 8 Trainium2 NeuronCores.

Algorithm (host side folds everything into matmul weights):
  out[b,:,n] = circconv_1024(S1 @ x1[b,:,n], S2 @ x2[b,:,n])
Decomposed via x^1024-1 = (x^512-1)(x^512+1):
  cyclic-512 branch (rFFT512) + negacyclic-512 branch (odd DFT), both fused
  with the count-sketch matrices into dense real forward matrices
  W_j [512c -> 1024 freq rows], applied as bf16 matmuls. Middle (complex
  multiply) runs on bf16 SBUF tiles on the vector engine. Inverse transforms
  are two block-diagonal [512 rows -> 512 outs] bf16 matmuls; the final
  unfold (c+d, c-d) is split between the vector and gpsimd engines reading
  inverse PSUM and writing packed bf16 output tiles per position tile.

v3 layout (current KCFG):
  - x1/x2 and all weights cast to bf16 on the HOST; x/weight loads are plain
    HWDGE transfers on the sync (SP) queue.
  - output staged in bf16 (host upcasts to f32 after gather; ~0.1% extra
    quantization, well inside the 2e-2 gate): halves store traffic and makes
    staged unfold ops pure-bf16 (2x DVE rate).
  - steady tiles: all 8 lo/hi outputs packed into ONE [128, 8*PT] bf16 tile,
    stored as a single SWDGE DMA on the Pool queue (25ns SEQ issue; descgen
    on the idle Pool engine) -> HWDGE/SP freed, DMA count per tile = 3.
  - unfold engine split: cs0/cs1 on Pool, rest on DVE; the 2 single-row
    DC/Nyquist cmult fixups on Pool (each costs a full row-time).
  - last tile: cs on Pool, act_d staging (Act drains psd -> no PE stall on
    PSUM banks; unfolds become 265ns bf16 ops), stores packed per-oc on the
    sync queue (scalar-queue stores would head-of-line-block the ds drains).
  - PE warm-up matmuls pin pe_busy_start early so real matmuls run at full
    p-state; head is DMA-pipeline-bound (x1 + first w piece ~4.7us).

Sharding: batch 32 -> 4 per core (data parallel), weights replicated.
Layout: channels/freq rows on SBUF partitions, positions on free axis.
No transposes anywhere.
"""
import sys

sys.path.insert(0, "/opt/trn_rl_repo")

import numpy as np
import ml_dtypes
import concourse.bass as bass
import concourse.mybir as mybir
from concourse import bacc
from concourse.tile import TileContext
from concourse.bass_utils import run_bass_kernel_spmd

B, C, HW, O = 32, 512, 784, 1024
NCORES = 8
BPC = B // NCORES  # 4 batches per core
PT = 392  # positions per tile (784 = 2*392; tiles never cross batch bounds)
NT = BPC * HW // PT  # 8 pos tiles per core
H = O // 2  # 512
F32, F32R, BF16 = mybir.dt.float32, mybir.dt.float32r, mybir.dt.bfloat16
BF16NP = ml_dtypes.bfloat16


def _build_host_matrices(sketch1, sketch2):
    """Fused fwd [512 c, 1024 freq-rows]; inverse IE/IF [256,256], ID [512,512].

    Level-2 folded row layout: e=rfft256, f=oddDFT256, d=oddDFT512; inverse
    weights carry the unfold 1/2 factors. All returned as bf16.
    """

    def build_fwd(sketch):
        sk = np.asarray(sketch, dtype=np.float64)
        Sp = sk[:H] + sk[H:]
        Sm = sk[:H] - sk[H:]
        Spp = Sp[:256] + Sp[256:]
        Spm = Sp[:256] - Sp[256:]
        n2 = np.arange(256)[None, :]
        k2 = np.arange(129)[:, None]
        Mc2 = np.exp(-2j * np.pi * k2 * n2 / 256) @ Spp
        k2f = np.arange(128)[:, None]
        Mo2 = np.exp(-2j * np.pi * n2 * (2 * k2f + 1) / 512) @ Spm
        n = np.arange(H)[None, :]
        ko = np.arange(256)[:, None]
        Mo = np.exp(-2j * np.pi * n * (2 * ko + 1) / O) @ Sm
        W = np.zeros((O, C))
        W[0:128] = Mc2[0:128].real
        W[128] = Mc2[128].real
        W[129:256] = Mc2[1:128].imag
        W[256:384] = Mo2.real
        W[384:512] = Mo2.imag
        W[512:768] = Mo.real
        W[768:1024] = Mo.imag
        return np.ascontiguousarray(W.T).astype(BF16NP)  # [C, O]

    j2 = np.arange(256)[None, :]
    k = np.arange(128)[:, None]
    IE = np.zeros((256, 256))
    IE[0:128] = 2 * np.cos(2 * np.pi * k * j2 / 256) / 256
    IE[0] = 1.0 / 256
    IE[128] = np.cos(np.pi * j2) / 256
    ki = np.arange(1, 128)[:, None]
    IE[129:256] = -2 * np.sin(2 * np.pi * ki * j2 / 256) / 256
    IF = np.zeros((256, 256))
    IF[0:128] = 2 * np.cos(2 * np.pi * (2 * k + 1) * j2 / 512) / 256
    IF[128:256] = -2 * np.sin(2 * np.pi * (2 * k + 1) * j2 / 512) / 256
    j = np.arange(H)[None, :]
    ko = np.arange(256)[:, None]
    ID = np.zeros((H, H))
    ID[0:256] = 2 * np.cos(2 * np.pi * (2 * ko + 1) * j / O) / H
    ID[256:512] = -2 * np.sin(2 * np.pi * (2 * ko + 1) * j / O) / H
    return (
        build_fwd(sketch1),
        build_fwd(sketch2),
        (IE / 4).astype(BF16NP),
        (IF / 4).astype(BF16NP),
        (ID / 2).astype(BF16NP),
    )


def _build_program(cfg=None):
    cfg = cfg or {}
    psf_bufs = cfg.get("psf_bufs", 2)
    pse_bufs = cfg.get("pse_bufs", 1)
    psq_bufs = cfg.get("psq_bufs", 1)
    psd_bufs = cfg.get("psd_bufs", 4)
    xbufs = cfg.get("xbufs", 2)
    fbufs = cfg.get("fbufs", 3)
    obufs = cfg.get("obufs", 2)
    # engine per unfold op [cs0..cs3, lo0,hi0,lo1,hi1,lo2,hi2,lo3,hi3]
    ueng = cfg.get("ueng", "ddpp" + "dp" * 4)
    store_split = cfg.get("store_split", 4)  # 1 or 4 pieces per tile
    warm = cfg.get("warm", 6)  # warm-up matmuls to ramp PE clock
    warm_ap = cfg.get("warm_ap", 392)
    x_eng = cfg.get("x_eng", "sync")  # HWDGE queue for x loads
    o_eng = cfg.get("o_eng", "scalar")  # queue for output stores
    es_skip = cfg.get("es_skip", False)  # cs reads pe_ PSUM directly
    tail_split = cfg.get("tail_split", False)
    x_first = cfg.get("x_first", True)  # lead DMA queues with tile-0 x loads
    nt_override = cfg.get("nt", NT)
    packed_oc = cfg.get("packed_oc", False)  # lo+hi in one tile, 1 DMA per oc
    act_f = cfg.get("act_f", None)  # drain pf_ PSUM->SBUF (needed for cs on pool)
    act_d = cfg.get("act_d", None)  # drain pd PSUM->SBUF (needed for lo/hi on pool)
    xsplit0 = cfg.get("xsplit0", False)  # split tile-0 x1 load per cc chunk
    warm_eng = cfg.get("warm_eng", "gpsimd")  # engine for warm tile memset
    # per-store queue pattern for the last tile (when not packed)
    store_eng_last = cfg.get("store_eng_last", None)

    nc = bacc.Bacc(None)
    x1e = nc.declare_dram_parameter("x1", [BPC, C, HW], BF16, isOutput=False)
    x2e = nc.declare_dram_parameter("x2", [BPC, C, HW], BF16, isOutput=False)
    w1e = nc.declare_dram_parameter("w1", [C, O], BF16, isOutput=False)
    w2e = nc.declare_dram_parameter("w2", [C, O], BF16, isOutput=False)
    iee = nc.declare_dram_parameter("ie", [256, 256], BF16, isOutput=False)
    ife = nc.declare_dram_parameter("if", [256, 256], BF16, isOutput=False)
    ide = nc.declare_dram_parameter("id", [H, H], BF16, isOutput=False)
    OD = BF16 if cfg.get("obf16") else F32
    oute = nc.declare_dram_parameter("out", [BPC, O, HW], OD, isOutput=True)

    ENG = {"sync": "sync", "scalar": "scalar", "gpsimd": "gpsimd", "vector": "vector"}
    xq = getattr(nc, ENG[x_eng])
    oq = getattr(nc, ENG[o_eng])

    with TileContext(nc) as tc:
        with (
            tc.tile_pool(name="wpool", bufs=1) as wpool,
            tc.tile_pool(name="xpool", bufs=xbufs) as xpool,
            tc.tile_pool(name="fpool", bufs=fbufs) as fpool,
            tc.tile_pool(name="opool", bufs=obufs) as opool,
            tc.tile_pool(name="psf", bufs=psf_bufs, space="PSUM") as psf,
            tc.tile_pool(name="pse", bufs=pse_bufs, space="PSUM") as pse,
            tc.tile_pool(name="psq", bufs=psq_bufs, space="PSUM") as psq,
            tc.tile_pool(name="psd", bufs=psd_bufs, space="PSUM") as psd,
        ):
            # ---- PE warm-up: ramp the tensor clock while DMAs land ----
            if warm:
                wa = wpool.tile([128, warm_ap], BF16, tag="warm_a", name="warm_a")
                getattr(nc, ENG[warm_eng]).memset(wa[:], 0.0)
                for wi in range(warm):
                    pw_ = psf.tile([128, warm_ap], F32, tag="psf", name=f"warm{wi}")
                    nc.tensor.matmul(
                        pw_[:], wa[:, 0:128], wa[:], start=True, stop=True
                    )

            def load_x(t, b, nsl, j_only=None, eng=None, ccsplit=False):
                pw = nsl.stop - nsl.start
                xr = {}
                for j, xe in ((1, x1e), (2, x2e)):
                    if j_only is not None and j != j_only:
                        continue
                    xt = xpool.tile([128, 4 * pw], BF16, tag=f"x{j}", name=f"x{j}_{t}")
                    if ccsplit:
                        k = 4 // int(ccsplit)
                        for g in range(int(ccsplit)):
                            c0, c1 = g * k, (g + 1) * k
                            (eng or xq).dma_start(
                                out=xt[:, c0 * pw : c1 * pw].rearrange(
                                    "p (c n) -> p c n", c=k
                                ),
                                in_=xe[b, c0 * 128 : c1 * 128, nsl].rearrange(
                                    "(c p) n -> p c n", c=k
                                ),
                            )
                    else:
                        (eng or xq).dma_start(
                            out=xt[:].rearrange("p (c n) -> p c n", c=4),
                            in_=xe[b, :, nsl].rearrange("(c p) n -> p c n", c=4),
                        )
                    xr[j] = xt
                return xr

            # ---- weights (already bf16 in DRAM; plain loads) ----
            w1r, w2r, iet, ift, idt = [], [], [], [], []
            specs = {
                "w1r": (w1r, w1e, O, 4),
                "w2r": (w2r, w2e, O, 4),
                "ie": (iet, iee, 256, 2),
                "if": (ift, ife, 256, 2),
                "id": (idt, ide, H, 4),
            }

            wsplit = cfg.get("wsplit", 4)  # load w1r/w2r in this many col pieces

            def make_w(nm):
                # one wide [128, 4*O] tile; chunk cc at free offset cc*O
                lst, ext, shp, nch = specs[nm]
                big = wpool.tile([128, nch * shp], BF16, tag=nm, name=nm)
                for cc in range(nch):
                    lst.append(big[:, cc * shp : (cc + 1) * shp])
                return big

            def load_w_piece(nm, s, ws):
                lst, ext, shp, nch = specs[nm]
                big = _wbig[nm]
                csl = slice(s * shp // ws, (s + 1) * shp // ws)
                nc.sync.dma_start(
                    out=big[:].rearrange("p (c n) -> p c n", c=nch)[:, :, csl],
                    in_=ext[:, csl].rearrange("(c p) n -> p c n", c=nch),
                )

            def load_w(nm):
                lst, ext, shp, nch = specs[nm]
                for cc in range(nch):
                    t = wpool.tile([128, shp], BF16, tag=f"{nm}{cc}", name=f"{nm}{cc}")
                    nc.sync.dma_start(out=t[:], in_=ext[cc * 128 : (cc + 1) * 128])
                    lst.append(t)

            _wbig = {"w1r": make_w("w1r"), "w2r": make_w("w2r")}
            # head order: x1(t0) -> first w1 pieces -> x2(t0) -> rest of w1 ->
            # w2 -> inverse weights, so the j=1 forward starts as early as
            # possible and each piece lands just ahead of its fc groups
            _xr_pre = {}
            _jp = cfg.get("job_pws")
            _pw0 = _jp[0][0] if _jp else PT
            _x0q = nc.gpsimd if cfg.get("x0_gpsimd") else nc.sync
            if x_first == "w0":
                # w1 piece 0 first (small; first fc group needs it), then x1,
                # then the rest: fc groups consume pieces slower than they land
                load_w_piece("w1r", 0, wsplit)
                if wsplit >= 8:
                    load_w_piece("w1r", 1, wsplit)
                _xr_pre[0] = load_x(0, 0, slice(0, _pw0), j_only=1, eng=_x0q)
                for s in range(2 if wsplit >= 8 else 1, wsplit):
                    load_w_piece("w1r", s, wsplit)
                _xr_pre[0].update(load_x(0, 0, slice(0, _pw0), j_only=2, eng=_x0q))
                for s in range(wsplit):
                    load_w_piece("w2r", s, wsplit)
            elif x_first:
                _xr_pre[0] = load_x(
                    0, 0, slice(0, _pw0), j_only=1, eng=_x0q, ccsplit=xsplit0
                )
                for s in range(min(2, wsplit)):
                    load_w_piece("w1r", s, wsplit)
                _xr_pre[0].update(load_x(0, 0, slice(0, _pw0), j_only=2, eng=_x0q))
                for s in range(min(2, wsplit), wsplit):
                    load_w_piece("w1r", s, wsplit)
                for s in range(wsplit):
                    load_w_piece("w2r", s, wsplit)
            else:
                for s in range(wsplit):
                    load_w_piece("w1r", s, wsplit)
                for s in range(wsplit):
                    load_w_piece("w2r", s, wsplit)
            load_w("ie")
            load_w("if")
            load_w("id")

            # ---- main loop over position tiles ----
            job_pws = cfg.get("job_pws")
            if job_pws:
                jobs = []
                for b in range(BPC):
                    n0 = 0
                    for pw in job_pws[b]:
                        jobs.append((len(jobs), b, n0, pw))
                        n0 += pw
                    assert n0 == HW
            else:
                jobs = [(t, (t // 2), (t % 2) * PT, PT) for t in range(nt_override)]
            if tail_split and nt_override == NT:
                ts = int(tail_split)
                lt, lb, ln0, _ = jobs.pop()
                for s in range(ts):
                    jobs.append((lt + s, lb, ln0 + s * PT // ts, PT // ts))
            dfirst = cfg.get("dfirst", False)
            fc_order = [4, 5, 6, 7, 0, 1, 2, 3] if dfirst else list(range(8))
            pair_d = [(4, 6), (5, 7)]
            pair_ef = [(0, 1), (2, 3)]

            last_t = jobs[-1][0]
            ueng_last = cfg.get("ueng_last", ueng)
            per_tile = cfg.get("per_tile", {})
            defer_store = cfg.get("defer_store", False)
            pending_store = []

            def flush_store():
                while pending_store:
                    ob, onsl, ot, opw = pending_store.pop(0)
                    oq.dma_start(
                        out=oute[ob].rearrange("(c q p) n -> p c q n", c=2, q=4)[
                            :, :, :, onsl
                        ],
                        in_=ot[:].rearrange("p (c q n) -> p c q n", c=2, q=4)[
                            :, :, :, 0:opw
                        ],
                    )

            for t, b, n0, pw in jobs:
                if defer_store:
                    flush_store()
                nsl = slice(n0, n0 + pw)
                xr = _xr_pre[t] if t in _xr_pre else load_x(t, b, nsl)
                W_ = slice(0, pw)
                ue = ueng_last if t == last_t else ueng
                ptc = per_tile.get(t) or per_tile.get(str(t)) or {}
                ue = ptc.get("ueng", ue)
                fft = {}
                prod = {}
                cch = []
                cs = [None] * 4

                def fwd(j, fcs, t=t, xr=xr, pw=pw, W_=W_, fft=fft):
                    wr = w1r if j == 1 else w2r
                    for fc in fcs:
                        ps = psf.tile([128, pw], F32, tag="psf", name=f"psf{j}_{fc}_{t}")
                        for cc in range(4):
                            nc.tensor.matmul(
                                ps[:, W_],
                                wr[cc][:, fc * 128 : (fc + 1) * 128],
                                xr[j][:, cc * pw : (cc + 1) * pw],
                                start=(cc == 0),
                                stop=(cc == 3),
                            )
                        ft = fpool.tile(
                            [128, pw], BF16, tag=f"fft{j}_{fc}", name=f"fft{j}_{fc}_{t}"
                        )
                        nc.scalar.copy(out=ft[:, W_], in_=ps[:, W_])
                        fft[(j, fc)] = ft

                cm_last = cfg.get("cmult_last", "dddd")

                def cmult(pairs, t=t, W_=W_, fft=fft, prod=prod, cm_last=cm_last):
                    # complex multiply (bf16, all-SBUF): chunk pairs (re,im)
                    for re_c, im_c in pairs:
                        pidx = {0: 0, 2: 1, 4: 2, 5: 3}[re_c]
                        eng = (
                            nc.gpsimd
                            if t == last_t and cm_last[pidx] == "p"
                            else nc.vector
                        )
                        a1, b1 = fft[(1, re_c)], fft[(1, im_c)]
                        a2, b2 = fft[(2, re_c)], fft[(2, im_c)]
                        m1 = fpool.tile([128, pw], BF16, tag="m1", name=f"m1_{re_c}_{t}")
                        m2 = fpool.tile([128, pw], BF16, tag="m2", name=f"m2_{re_c}_{t}")
                        pr = fpool.tile(
                            [128, pw], BF16, tag=f"pr{re_c}", name=f"pr{re_c}_{t}"
                        )
                        pi = fpool.tile(
                            [128, pw], BF16, tag=f"pi{im_c}", name=f"pi{im_c}_{t}"
                        )
                        _cms = (
                            cfg.get("cm_split_last", ())
                            if t == last_t
                            else cfg.get("cm_split", ())
                        )
                        if re_c in _cms:
                            # pr-chain on `eng`, independent pi-chain on Pool
                            m3 = fpool.tile(
                                [128, pw], BF16, tag="m3", name=f"m3_{re_c}_{t}"
                            )
                            m4 = fpool.tile(
                                [128, pw], BF16, tag="m4", name=f"m4_{re_c}_{t}"
                            )
                            eng.tensor_mul(m1[:, W_], a1[:, W_], a2[:, W_])
                            eng.tensor_mul(m2[:, W_], b1[:, W_], b2[:, W_])
                            eng.tensor_sub(pr[:, W_], m1[:, W_], m2[:, W_])
                            nc.gpsimd.tensor_mul(m3[:, W_], a1[:, W_], b2[:, W_])
                            nc.gpsimd.tensor_mul(m4[:, W_], b1[:, W_], a2[:, W_])
                            nc.gpsimd.tensor_add(pi[:, W_], m3[:, W_], m4[:, W_])
                        else:
                            eng.tensor_mul(m1[:, W_], a1[:, W_], a2[:, W_])
                            eng.tensor_mul(m2[:, W_], b1[:, W_], b2[:, W_])
                            eng.tensor_sub(pr[:, W_], m1[:, W_], m2[:, W_])
                            eng.tensor_mul(m1[:, W_], a1[:, W_], b2[:, W_])
                            eng.tensor_mul(m2[:, W_], b1[:, W_], a2[:, W_])
                            eng.tensor_add(pi[:, W_], m1[:, W_], m2[:, W_])
                        if re_c == 0:
                            # row 0 of the (0,1) pair: DC_e (re) and Nyquist-256
                            # (held in im slot row 0) are real-only products
                            feng = (
                                nc.gpsimd if cfg.get("fix_eng") == "p" else eng
                            )
                            feng.tensor_mul(pr[0:1, W_], a1[0:1, W_], a2[0:1, W_])
                            feng.tensor_mul(pi[0:1, W_], b1[0:1, W_], b2[0:1, W_])
                        prod[re_c] = pr
                        prod[im_c] = pi

                act_stage = cfg.get("act_stage", False) or (
                    t == last_t and cfg.get("act_stage_last", False)
                )
                act_f_t = act_f if act_f is not None else act_stage
                act_d_t = act_d if act_d is not None else act_stage
                if t == last_t and cfg.get("act_stage_last", False):
                    act_f_t = act_d_t = True
                act_f_t = ptc.get("act_f", act_f_t)
                act_d_t = ptc.get("act_d", act_d_t)

                def inv_ef(t=t, W_=W_, prod=prod, cch=cch, cs=cs, ue=ue,
                           act_f_t=act_f_t):
                    # inverse level2: e,f [256] then c = unfold2(e,f) in SBUF
                    for oc2 in range(2):
                        osl2 = slice(oc2 * 128, (oc2 + 1) * 128)
                        pe_ = pse.tile([128, pw], F32, tag="pse", name=f"pse{oc2}_{t}")
                        pf_ = psq.tile([128, pw], F32, tag="psq", name=f"psq{oc2}_{t}")
                        for rc in range(2):
                            nc.tensor.matmul(
                                pe_[:, W_], iet[rc][:, osl2], prod[rc][:, W_],
                                start=(rc == 0), stop=(rc == 1),
                            )
                        for rc in range(2):
                            nc.tensor.matmul(
                                pf_[:, W_], ift[rc][:, osl2], prod[2 + rc][:, W_],
                                start=(rc == 0), stop=(rc == 1),
                            )
                        es = opool.tile(
                            [128, pw], OD, tag=f"es{oc2}", name=f"es{oc2}_{t}"
                        )
                        nc.scalar.copy(out=es[:, W_], in_=pe_[:, W_])
                        if act_f_t:
                            # drain pf_ to SBUF too so unfolds are SBUF-only
                            # (GPSIMD cannot touch PSUM) and psq frees fast
                            fs = opool.tile(
                                [128, pw], OD, tag=f"fs{oc2}", name=f"fs{oc2}_{t}"
                            )
                            nc.scalar.copy(out=fs[:, W_], in_=pf_[:, W_])
                            cch.append((es, fs))
                        else:
                            cch.append((es, pf_))
                    for oc in range(4):
                        es, pf_ = cch[oc % 2]
                        ct = opool.tile([128, pw], OD, tag=f"c{oc}", name=f"c{oc}_{t}")
                        ceng = nc.gpsimd if ue[oc] == "p" else nc.vector
                        if oc < 2:
                            ceng.tensor_add(ct[:, W_], es[:, W_], pf_[:, W_])
                        else:
                            ceng.tensor_sub(ct[:, W_], es[:, W_], pf_[:, W_])
                        cs[oc] = ct

                pds = [None] * 4

                def inv_d(ocs, rc_order=(0, 1, 2, 3), t=t, W_=W_, prod=prod, pds=pds,
                          act_d_t=act_d_t):
                    spread = ptc.get("spread_dbanks", cfg.get("spread_dbanks", False))
                    for oc in ocs:
                        osl = slice(oc * 128, (oc + 1) * 128)
                        if spread and oc == 2:
                            pd = psq.tile([128, pw], F32, tag="psq", name=f"psd{oc}_{t}")
                        elif spread and oc == 3:
                            pd = pse.tile([128, pw], F32, tag="pse", name=f"psd{oc}_{t}")
                        else:
                            pd = psd.tile([128, pw], F32, tag="psd", name=f"psd{oc}_{t}")
                        for i, rc in enumerate(rc_order):
                            nc.tensor.matmul(
                                pd[:, W_], idt[rc][:, osl], prod[4 + rc][:, W_],
                                start=(i == 0), stop=(i == 3),
                            )
                        if act_d_t:
                            ds = opool.tile(
                                [128, pw], OD, tag=f"ds{oc}", name=f"ds{oc}_{t}"
                            )
                            nc.scalar.copy(out=ds[:, W_], in_=pd[:, W_])
                            pds[oc] = ds
                        else:
                            pds[oc] = pd

                otile = [None]

                def unfold(ocs, t=t, b=b, nsl=nsl, W_=W_, cs=cs, pds=pds, ue=ue,
                           pw=pw, n0=n0, otile=otile):
                    for oc in ocs:
                        osl = slice(oc * 128, (oc + 1) * 128)
                        leng = nc.gpsimd if ue[4 + 2 * oc] == "p" else nc.vector
                        heng = nc.gpsimd if ue[5 + 2 * oc] == "p" else nc.vector
                        if t == last_t and cfg.get("packed_oc_last") == "oc":
                            lohi = opool.tile(
                                [128, 2 * pw], OD, tag=f"lh{oc}", name=f"lh{oc}_{t}"
                            )
                            lo = lohi[:, 0:pw]
                            hi = lohi[:, pw : 2 * pw]
                            leng.tensor_add(lo, cs[oc][:, W_], pds[oc][:, W_])
                            heng.tensor_sub(hi, cs[oc][:, W_], pds[oc][:, W_])
                            sq = (
                                getattr(nc, ENG[store_eng_last[oc]])
                                if store_eng_last
                                else oq
                            )
                            sq.dma_start(
                                out=oute[b].rearrange(
                                    "(c q p) n -> q p c n", c=2, q=4
                                )[oc][:, :, nsl],
                                in_=lohi[:].rearrange("p (c n) -> p c n", c=2),
                            )
                            continue
                        if packed_oc == "full" and not (t == last_t and store_eng_last):
                            if otile[0] is None:
                                otile[0] = opool.tile(
                                    [128, 8 * pw], OD, tag="ot", name=f"ot_{t}"
                                )
                            ot = otile[0]
                            lo = ot[:, oc * pw : (oc + 1) * pw]
                            hi = ot[:, (4 + oc) * pw : (5 + oc) * pw]
                            leng.tensor_add(lo, cs[oc][:, W_], pds[oc][:, W_])
                            heng.tensor_sub(hi, cs[oc][:, W_], pds[oc][:, W_])
                            if oc == 3:
                                if defer_store:
                                    pending_store.append((b, nsl, ot, pw))
                                else:
                                    oq.dma_start(
                                        out=oute[b].rearrange(
                                            "(c q p) n -> p c q n", c=2, q=4
                                        )[:, :, :, nsl],
                                        in_=ot[:].rearrange(
                                            "p (c q n) -> p c q n", c=2, q=4
                                        )[:, :, :, 0:pw],
                                    )
                            continue
                        if packed_oc and not (t == last_t and store_eng_last):
                            lohi = opool.tile(
                                [128, 2 * pw], OD, tag=f"lh{oc}", name=f"lh{oc}_{t}"
                            )
                            lo = lohi[:, 0:pw]
                            hi = lohi[:, pw : 2 * pw]
                            leng.tensor_add(lo, cs[oc][:, W_], pds[oc][:, W_])
                            heng.tensor_sub(hi, cs[oc][:, W_], pds[oc][:, W_])
                            oq.dma_start(
                                out=oute[b].rearrange(
                                    "(c q p) n -> q p c n", c=2, q=4
                                )[oc][:, :, nsl],
                                in_=lohi[:].rearrange("p (c n) -> p c n", c=2)[
                                    :, :, 0:pw
                                ],
                            )
                        else:
                            lot = opool.tile(
                                [128, pw], OD, tag=f"lo{oc}", name=f"lo{oc}_{t}"
                            )
                            hit = opool.tile(
                                [128, pw], OD, tag=f"hi{oc}", name=f"hi{oc}_{t}"
                            )
                            lo, hi = lot[:, W_], hit[:, W_]
                            leng.tensor_add(lo, cs[oc][:, W_], pds[oc][:, W_])
                            heng.tensor_sub(hi, cs[oc][:, W_], pds[oc][:, W_])
                            if t == last_t and store_eng_last:
                                lq = getattr(nc, ENG[store_eng_last[2 * oc]])
                                hq = getattr(nc, ENG[store_eng_last[2 * oc + 1]])
                            else:
                                lq, hq = oq, oq
                            lq.dma_start(out=oute[b, osl, nsl], in_=lo)
                            hq.dma_start(
                                out=oute[
                                    b, slice(512 + oc * 128, 512 + (oc + 1) * 128), nsl
                                ],
                                in_=hi,
                            )

                ilv = cfg.get("ilv", "j2")
                if t == last_t and cfg.get("ilv_last") is not None:
                    ilv = cfg.get("ilv_last")
                ilv = ptc.get("ilv", ilv)
                if ilv:
                    # emit each complex multiply as soon as its pair of chunks
                    # is transformed, so the tail inverse never waits on a
                    # long cmult chain
                    emit = {1: [(0, 1)], 3: [(2, 3)], 6: [(4, 6)], 7: [(5, 7)]}
                    if ilv == "j2":
                        fwd(1, [0, 1, 2, 3, 4, 6, 5, 7])
                        for fc in [0, 1, 2, 3, 4, 6, 5, 7]:
                            fwd(2, [fc])
                            if fc in emit:
                                cmult(emit[fc])
                    elif ilv == "alt2":
                        # ef chunks + their cmults + inv_ef FIRST, so the only
                        # PE work after the last fwd mm is inv_d (short tail)
                        for fc in [0, 1, 2, 3]:
                            fwd(1, [fc])
                            fwd(2, [fc])
                            if fc in emit:
                                cmult(emit[fc])
                        inv_ef()
                        for fc in [4, 6, 5, 7]:
                            fwd(1, [fc])
                            fwd(2, [fc])
                            if fc in emit:
                                cmult(emit[fc])
                    else:
                        for fc in [0, 1, 2, 3, 4, 6, 5, 7]:
                            fwd(1, [fc])
                            fwd(2, [fc])
                            if fc in emit:
                                cmult(emit[fc])
                    if ilv != "alt2":
                        inv_ef()
                    for oc in range(4):
                        inv_d([oc], rc_order=(0, 2, 1, 3))
                        unfold([oc])
                elif dfirst:
                    fwd(1, fc_order)
                    fwd(2, fc_order)
                    cmult(pair_d)
                    inv_d([0, 1, 2, 3])
                    cmult(pair_ef)
                    inv_ef()
                    unfold([0, 1, 2, 3])
                else:
                    fwd(1, fc_order)
                    fwd(2, fc_order)
                    cmult(pair_ef + pair_d)
                    inv_ef()
                    for oc in range(4):
                        inv_d([oc])
                        unfold([oc])
            if defer_store:
                flush_store()

    nc.finalize()
    return nc


_NC_CACHE = None
_NC_CFG = None
KCFG = {
    "o_eng": "gpsimd",
    "x_eng": "sync",
    "psf_bufs": 3,
    "psd_bufs": 3,
    "ueng": "ppdddddddddd",
    "packed_oc": "full",
    "obf16": True,
    "wsplit": 4,
    "act_f": True,
    "act_d": False,
    "ilv": False,
    "ilv_last": "alt",
    "act_stage_last": False,
    "ueng_last": "dddddddddddd",
    "cmult_last": "dddd",
    "store_eng_last": ["sync", "sync", "sync", "sync"],
    "packed_oc_last": "oc",
    "per_tile": {"7": {"ueng": "dddddddddddd", "act_d": True}},
    "fix_eng": "p",
}


def _make_in_maps(x1, x2, sketch1, sketch2):
    w1, w2, ie, if_, idm = _build_host_matrices(sketch1, sketch2)
    x1f = np.asarray(x1, dtype=np.float32).reshape(B, C, HW).astype(BF16NP)
    x2f = np.asarray(x2, dtype=np.float32).reshape(B, C, HW).astype(BF16NP)
    in_maps = []
    for i in range(NCORES):
        bs = slice(i * BPC, (i + 1) * BPC)
        in_maps.append(
            {
                "x1": np.ascontiguousarray(x1f[bs]),
                "x2": np.ascontiguousarray(x2f[bs]),
                "w1": w1,
                "w2": w2,
                "ie": ie,
                "if": if_,
                "id": idm,
            }
        )
    return in_maps


def kernel(x1, x2, sketch1, sketch2):
    global _NC_CACHE, _NC_CFG
    if _NC_CACHE is None or _NC_CFG != KCFG:
        _NC_CACHE = _build_program(KCFG)
        _NC_CFG = dict(KCFG)
    nc = _NC_CACHE
    in_maps = _make_in_maps(x1, x2, sketch1, sketch2)
    res = run_bass_kernel_spmd(nc, in_maps, list(range(NCORES)))
    out = np.concatenate([res.results[i]["out"] for i in range(NCORES)], axis=0)
    return out.reshape(B, O, 28, 28).astype(np.float32)


if __name__ == "__main__":
    rng = np.random.default_rng(0)
    x1 = rng.standard_normal((B, C, 28, 28)).astype(np.float32)
    x2 = rng.standard_normal((B, C, 28, 28)).astype(np.float32)
    h1 = rng.integers(0, O, C)
    s1 = rng.integers(0, 2, C) * 2.0 - 1.0
    h2 = rng.integers(0, O, C)
    s2 = rng.integers(0, 2, C) * 2.0 - 1.0
    sk1 = np.zeros((O, C), np.float32)
    sk1[h1, np.arange(C)] = s1
    sk2 = np.zeros((O, C), np.float32)
    sk2[h2, np.arange(C)] = s2
    got = kernel(x1, x2, sk1, sk2)
    p1 = np.einsum("bchw,oc->bohw", x1, sk1).reshape(B, O, HW)
    p2 = np.einsum("bchw,oc->bohw", x2, sk2).reshape(B, O, HW)
    ref = np.fft.ifft(np.fft.fft(p1, axis=1) * np.fft.fft(p2, axis=1), axis=1).real
    err = np.abs(got.reshape(B, O, HW) - ref).max() / np.abs(ref).max()
    print("self-test max rel err:", err)

